# revision 30
# baseline (speedup 1.0000x reference)
"""Grouped Query Attention on 8 TRN2 NeuronCores (v7).

Sharding: batch x s_q-quarter (core c -> batch c//4, query rows
[512*(c%4), 512*(c%4+1))). Each core computes the Q projection for its
512 query rows, attention for all 16 heads over its query rows, and the
output projection for a disjoint [512, 2048] slice of the output.

KV: each core projects K^T and V for its OWN sequence quarter, packs
them into DRAM (per-piece, right after each evacuation), and two
4-core AllGathers (K first, then V) assemble the full K^T/V while the
tensor engine runs the Q projection.

v7 structure:
- Phase-1 PE order K -> Q0-3 -> V -> Q4-15 so the V weights (second
  half of wkv) have time to land; wkv K-columns stream first.
- DMA queues: consts+x+wq(even) on sync, wkv+packs+K-gathers on
  scalar, wq(odd)+V-gathers on gpsimd; all wq into fresh buffers (no
  WAR waits blocking the collective doorbells).
- Phase 2 is one flat software pipeline over 128 (pair, k-tile) steps:
  scores+exp (A) stream with attnV (B) lagging 3 steps; no prologue
  stall and no per-pair boundary stalls.
- Softmax denominator per pair: DVE tree-add over the 16 k-tiles +
  GPSIMD partition_all_reduce; reciprocal+normalize deferred ~1 pair.
  The last pair takes a short path (half-trees + PE ones-matmul +
  broadcast) so phase 3 is not held up.
- Bias adds via DVE tensor_add with pre-broadcast bias rows; 1/sqrt(d)
  folded into Wq on host.
"""

import numpy as np

E = 2048
S = 2048
P = 128
H = 16
G = 4
SQ = 512          # query rows per core
EB = E // P       # 16 e-blocks (contraction tiles)
NCORES = 8

_NC = None
TRACE = False
LAST_RESULT = None


def _build():
    import concourse.bacc as bacc
    import concourse.mybir as mybir
    import concourse.tile as tile
    from concourse import bass_isa

    f32 = mybir.dt.float32
    bf16 = mybir.dt.bfloat16
    EXP = mybir.ActivationFunctionType.Exp
    IDENT = mybir.ActivationFunctionType.Identity

    nc = bacc.Bacc("TRN2", target_bir_lowering=False, debug=False,
                   num_devices=NCORES)

    xt = nc.declare_dram_parameter("xt", [P, EB, SQ], bf16, isOutput=False).ap()
    wq = nc.declare_dram_parameter("wq", [H, P, EB, P], bf16, isOutput=False).ap()
    wkv = nc.declare_dram_parameter("wkv", [P, EB, 2 * E // G], bf16, isOutput=False).ap()
    wo = nc.declare_dram_parameter("wo", [P, EB, E], bf16, isOutput=False).ap()
    bq = nc.declare_dram_parameter("bq", [P, H], f32, isOutput=False).ap()
    bkvk = nc.declare_dram_parameter("bkvk", [P, 4], f32, isOutput=False).ap()
    bkvv = nc.declare_dram_parameter("bkvv", [1, 512], f32, isOutput=False).ap()
    bo = nc.declare_dram_parameter("bo", [1, E], f32, isOutput=False).ap()
    out = nc.declare_dram_parameter("out", [SQ, E], f32, isOutput=True).ap()

    RG = [[0, 1, 2, 3], [4, 5, 6, 7]]

    with tile.TileContext(nc) as tc:
        with tc.tile_pool(name="consts", bufs=1) as cp, \
             tc.tile_pool(name="qtsp", bufs=1) as qtsp, \
             tc.tile_pool(name="kvp", bufs=1) as kvp, \
             tc.tile_pool(name="otp", bufs=1) as otp, \
             tc.tile_pool(name="dram", bufs=1, space="DRAM") as dp:
            # consts on the sync queue, ahead of x (tiny transfers)
            bq_s = cp.tile([P, H], f32, tag="bqs")
            nc.sync.dma_start(bq_s, bq)
            bkvk_s = cp.tile([P, 4], f32, tag="bkvks")
            nc.sync.dma_start(bkvk_s, bkvk)
            bkvv_b = cp.tile([P, 512], f32, tag="bkvvb")
            bo_b = cp.tile([P, E], f32, tag="bob")
            onec = cp.tile([P, 1], bf16, tag="onec")
            nc.vector.memset(onec, 1.0)

            qts = qtsp.tile([P, H, SQ], bf16, tag="qts")    # Q^T, [hd, head, sq]
            kts = kvp.tile([P, G, S], bf16, tag="kts")      # K^T, [hd, group, sk]
            vgs = kvp.tile([P, EB, 512], bf16, tag="vgs")   # V, [sk, sk_tile, g*128+hd]
            OT = otp.tile([P, H, SQ], bf16, tag="ot")       # attn out, [hd, head, sq]

            kvkown = dp.tile([P, 4, 512], bf16, tag="kvkown")
            kvkall = dp.tile([4, P, 4, 512], bf16, tag="kvkall")
            kvvown = dp.tile([P, 4, 512], bf16, tag="kvvown")
            kvvall = dp.tile([4, P, 4, 512], bf16, tag="kvvall")

            # ---- Phase 1: projections from the SBUF-resident x^T.
            with tc.tile_pool(name="xsp", bufs=1) as xsp, \
                 tc.tile_pool(name="wkvp", bufs=1) as wkvp, \
                 tc.tile_pool(name="kvsg", bufs=1) as kvsg, \
                 tc.tile_pool(name="wqp", bufs=16) as wqp, \
                 tc.tile_pool(name="ps1", bufs=2, space="PSUM") as ps1, \
                 tc.tile_pool(name="ps1b", bufs=2, space="PSUM") as ps1b:
                # wkv on the scalar queue, K columns first in small
                # chunks: the K proj -> collective path is critical
                wkv_s = wkvp.tile([P, EB, 2 * E // G], bf16, tag="wkvs")
                KC = 2 * E // G
                for c4 in range(4):
                    nc.scalar.dma_start(wkv_s[:, 4 * c4:4 * (c4 + 1), 0:512],
                                        wkv[:, 4 * c4:4 * (c4 + 1), 0:512])
                nc.scalar.dma_start(wkv_s[:, 0:8, 512:KC], wkv[:, 0:8, 512:KC])
                nc.scalar.dma_start(wkv_s[:, 8:16, 512:KC], wkv[:, 8:16, 512:KC])
                # bias staging rows; broadcasts run on gpsimd
                bkvv_s = kvsg.tile([1, 512], f32, tag="bkvvs")
                nc.sync.dma_start(bkvv_s, bkvv)
                bo_s = kvsg.tile([1, E], f32, tag="bos")
                nc.sync.dma_start(bo_s, bo)

                # x on sync; wq split sync(even)/gpsimd(odd), all into
                # fresh buffers (no WAR waits on the gpsimd queue)
                xs = xsp.tile([P, EB, SQ], bf16, tag="xs")
                nc.sync.dma_start(xs[:, 0:2], xt[:, 0:2])
                nc.sync.dma_start(xs[:, 2:4], xt[:, 2:4])
                nc.sync.dma_start(xs[:, 4:10], xt[:, 4:10])
                nc.sync.dma_start(xs[:, 10:16], xt[:, 10:16])
                wq_s = []
                for m in range(H):
                    wqm = wqp.tile([P, EB, P], bf16, tag="wqm")
                    if m % 2 == 0:
                        nc.sync.dma_start(wqm, wq[m])
                    else:
                        nc.gpsimd.dma_start(wqm, wq[m])
                    wq_s.append(wqm)
                nc.gpsimd.partition_broadcast(bkvv_b, bkvv_s)
                nc.gpsimd.partition_broadcast(bo_b, bo_s)

                kvstg_k = kvsg.tile([P, 4, 512], bf16, tag="kvstgk")
                kvstg_v = kvsg.tile([P, 4, 512], bf16, tag="kvstgv")

                def q_head(m):
                    ps = ps1.tile([P, SQ], f32, tag="ps")
                    for b in range(EB):
                        nc.tensor.matmul(ps, wq_s[m][:, b], xs[:, b],
                                         start=(b == 0), stop=(b == EB - 1))
                    nc.vector.tensor_scalar_add(qts[:, m], ps,
                                                bq_s[:, m:m + 1])

                # K^T for all 4 groups; pack each piece immediately
                for m in range(G):
                    ps = ps1b.tile([P, 512], f32, tag="ps")
                    for b in range(EB):
                        nc.tensor.matmul(
                            ps, wkv_s[:, b, m * P:(m + 1) * P], xs[:, b],
                            start=(b == 0), stop=(b == EB - 1))
                    nc.scalar.activation(kvstg_k[:, m], ps, IDENT,
                                         bias=bkvk_s[:, m:m + 1])
                    nc.scalar.dma_start(kvkown[:, m], kvstg_k[:, m])
                nc.gpsimd.collective_compute(
                    "AllGather", mybir.AluOpType.bypass,
                    replica_groups=RG, ins=[kvkown[:]], outs=[kvkall[:]])

                # a few Q heads while the V weights finish landing
                for m in range(4):
                    q_head(m)

                # V in [s, d] orientation; per-piece packs
                for t in range(4):
                    ps = ps1b.tile([P, 512], f32, tag="ps")
                    for b in range(EB):
                        nc.tensor.matmul(
                            ps, xs[:, b, t * P:(t + 1) * P],
                            wkv_s[:, b, 512:KC],
                            start=(b == 0), stop=(b == EB - 1))
                    nc.vector.tensor_add(kvstg_v[:, t], ps, bkvv_b)
                    nc.scalar.dma_start(kvvown[:, t], kvstg_v[:, t])
                nc.gpsimd.collective_compute(
                    "AllGather", mybir.AluOpType.bypass,
                    replica_groups=RG, ins=[kvvown[:]], outs=[kvvall[:]])

                # per-source-core contiguous gathers: K on scalar,
                # V on gpsimd (scalar later carries the exp stream)
                for c in range(4):
                    nc.scalar.dma_start(
                        kts[:, :, 512 * c:512 * (c + 1)], kvkall[c])
                for c in range(4):
                    nc.gpsimd.dma_start(vgs[:, 4 * c:4 * (c + 1)],
                                        kvvall[c])

                for m in range(4, H):
                    q_head(m)

            # ---- Phase 2: one flat A/B software pipeline.
            with tc.tile_pool(name="wop", bufs=2) as wop, \
                 tc.tile_pool(name="eap", bufs=2) as eap, \
                 tc.tile_pool(name="r1p", bufs=1) as r1p, \
                 tc.tile_pool(name="rsp", bufs=1) as rsp, \
                 tc.tile_pool(name="dbp", bufs=1) as dbp, \
                 tc.tile_pool(name="rbp", bufs=1) as rbp, \
                 tc.tile_pool(name="psop", bufs=4, space="PSUM") as psop:
                won0 = wop.tile([P, EB, 512], bf16, tag="won")
                nc.sync.dma_start(won0, wo[:, :, 0:512])  # prefetch ph3

                recs = [None] * 8
                psos = [None] * 8
                eas = [None] * 8

                def denom_front(p):
                    # DVE tree over the 16 k-tiles -> gpsimd all-reduce
                    ea = eas[p]
                    r1 = r1p.tile([P, 8, 2, SQ], bf16, tag="r1")
                    nc.vector.tensor_add(r1, ea[:, 0:8], ea[:, 8:16])
                    nc.vector.tensor_add(r1[:, 0:4], r1[:, 0:4], r1[:, 4:8])
                    nc.vector.tensor_add(r1[:, 0:2], r1[:, 0:2], r1[:, 2:4])
                    rs = rsp.tile([P, 2, SQ], bf16, tag="rs")
                    nc.vector.tensor_add(rs, r1[:, 0], r1[:, 1])
                    den = dbp.tile([P, 2, SQ], f32, tag="den")
                    nc.gpsimd.partition_all_reduce(
                        den, rs, channels=P,
                        reduce_op=bass_isa.ReduceOp.add)
                    rec = rbp.tile([P, 2, SQ], f32, tag="rec")
                    recs[p] = (den, rec)

                def denom_back(p):
                    den, rec = recs[p]
                    nc.vector.reciprocal_approx_fast(rec, den)
                    pso0, pso1 = psos[p]
                    nc.vector.tensor_mul(OT[:, 2 * p], pso0, rec[:, 0])
                    nc.vector.tensor_mul(OT[:, 2 * p + 1], pso1, rec[:, 1])

                def denom7_half(r1, ea, lo):
                    nc.vector.tensor_add(r1[:, lo:lo + 2],
                                         ea[:, 2 * lo:2 * lo + 2],
                                         ea[:, 2 * lo + 2:2 * lo + 4])
                    nc.vector.tensor_add(r1[:, lo:lo + 1],
                                         r1[:, lo:lo + 1],
                                         r1[:, lo + 1:lo + 2])

                pscp_cm = tc.tile_pool(name="pscp", bufs=2, space="PSUM")
                pscp = pscp_cm.__enter__()
                LAG = 16
                r17 = None
                for i in range(128 + LAG):
                    if i < 128:
                        p, t = divmod(i, EB)
                        if t == 0:
                            ea_new = eap.tile([P, EB, 2, SQ], bf16, tag="ea")
                            eas[p] = ea_new
                        ps2 = pscp.tile([P, 2, SQ], f32, tag="ps2")
                        nc.tensor.matmul(ps2[:, 0],
                                         kts[:, p // 2, t * P:(t + 1) * P],
                                         qts[:, 2 * p], start=True, stop=True)
                        nc.tensor.matmul(ps2[:, 1],
                                         kts[:, p // 2, t * P:(t + 1) * P],
                                         qts[:, 2 * p + 1], start=True,
                                         stop=True)
                        nc.scalar.activation(eas[p][:, t], ps2, EXP)
                    if i >= LAG:
                        j = i - LAG
                        pb, tb = divmod(j, EB)
                        if tb == 0:
                            pso0 = psop.tile([P, SQ], f32, tag="pso")
                            pso1 = psop.tile([P, SQ], f32, tag="pso")
                            psos[pb] = (pso0, pso1)
                        pso0, pso1 = psos[pb]
                        nc.tensor.matmul(
                            pso0, vgs[:, tb, (pb // 2) * P:(pb // 2 + 1) * P],
                            eas[pb][:, tb, 0],
                            start=(tb == 0), stop=(tb == EB - 1))
                        nc.tensor.matmul(
                            pso1, vgs[:, tb, (pb // 2) * P:(pb // 2 + 1) * P],
                            eas[pb][:, tb, 1],
                            start=(tb == 0), stop=(tb == EB - 1))
                    # deferred denominator work, keyed to pipeline position
                    q16, r16 = divmod(i, EB)
                    if r16 == 2 and 1 <= q16 <= 7:
                        denom_front(q16 - 1)
                    if r16 == 1 and 2 <= q16 <= 8:
                        denom_back(q16 - 2)
                    if i == 123:
                        r17 = r1p.tile([P, 8, 2, SQ], bf16, tag="r1")
                        denom7_half(r17, eas[7], 0)
                        denom7_half(r17, eas[7], 2)
                        nc.vector.tensor_add(r17[:, 0], r17[:, 0], r17[:, 2])
                pscp_cm.__exit__(None, None, None)

                # epilogue: pair 7 short-path normalization
                denom7_half(r17, eas[7], 4)
                denom7_half(r17, eas[7], 6)
                nc.vector.tensor_add(r17[:, 4], r17[:, 4], r17[:, 6])
                rs7 = rsp.tile([P, 2, SQ], bf16, tag="rs")
                nc.vector.tensor_add(rs7, r17[:, 0], r17[:, 4])
                psl0 = psop.tile([1, 512], f32, tag="pso")
                nc.tensor.matmul(psl0, onec, rs7[:, 0], start=True, stop=True)
                psl1 = psop.tile([1, 512], f32, tag="pso")
                nc.tensor.matmul(psl1, onec, rs7[:, 1], start=True, stop=True)
                rec7s = dbp.tile([1, 2, 512], f32, tag="den")
                nc.vector.reciprocal_approx_fast(rec7s[:, 0], psl0)
                nc.vector.reciprocal_approx_fast(rec7s[:, 1], psl1)
                rec7b = rbp.tile([P, 2, 512], f32, tag="rec")
                nc.gpsimd.partition_broadcast(rec7b, rec7s)
                pso0, pso1 = psos[7]
                nc.vector.tensor_mul(OT[:, 14], pso0, rec7b[:, 0])
                nc.vector.tensor_mul(OT[:, 15], pso1, rec7b[:, 1])

                # ---- Phase 3: output projection, contraction over the
                # 16 head blocks; bias added on the PSUM->SBUF copy.
                with tc.tile_pool(name="obp", bufs=2) as obp, \
                     tc.tile_pool(name="ps3", bufs=2, space="PSUM") as ps3p:
                    wons = [won0]
                    for n in range(4):
                        if n + 1 < 4:
                            wnx = wop.tile([P, EB, 512], bf16, tag="won")
                            nc.sync.dma_start(
                                wnx, wo[:, :, 512 * (n + 1):512 * (n + 2)])
                            wons.append(wnx)
                        won = wons[n]
                        for ms in range(4):
                            ps = ps3p.tile([P, 512], f32, tag="ps")
                            for k in range(EB):
                                nc.tensor.matmul(
                                    ps, OT[:, k, ms * P:(ms + 1) * P],
                                    won[:, k],
                                    start=(k == 0), stop=(k == EB - 1))
                            ob = obp.tile([P, 512], f32, tag="ob")
                            nc.vector.tensor_add(
                                ob, ps, bo_b[:, 512 * n:512 * (n + 1)])
                            nc.sync.dma_start(
                                out[ms * P:(ms + 1) * P,
                                    512 * n:512 * (n + 1)], ob)

    nc.compile()
    return nc


def _get_nc():
    global _NC
    if _NC is None:
        _NC = _build()
    return _NC


def kernel(x, Wq, bq, Wkv, bkv, Wo, bo):
    from concourse.bass_utils import run_bass_kernel_spmd
    import ml_dtypes
    global LAST_RESULT

    bf = ml_dtypes.bfloat16
    x = np.asarray(x, np.float32)
    Wq = np.asarray(Wq, np.float32)
    bq = np.asarray(bq, np.float32)
    Wkv = np.asarray(Wkv, np.float32)
    bkv = np.asarray(bkv, np.float32)
    Wo = np.asarray(Wo, np.float32)
    bo = np.asarray(bo, np.float32)

    nc = _get_nc()
    sc = 1.0 / np.sqrt(E // H)
    wq_h = np.ascontiguousarray(
        (Wq * sc).reshape(EB, P, H, P).transpose(2, 1, 0, 3)).astype(bf)
    kcols = np.concatenate([Wkv[:, 256 * g:256 * g + 128] for g in range(G)], axis=1)
    vcols = np.concatenate([Wkv[:, 256 * g + 128:256 * g + 256] for g in range(G)], axis=1)
    wkv_re = np.concatenate([kcols, vcols], axis=1)  # [E, 1024]
    wkv_h = np.ascontiguousarray(
        wkv_re.reshape(EB, P, 2 * E // G).transpose(1, 0, 2)).astype(bf)
    wo_h = np.ascontiguousarray(Wo.reshape(EB, P, E).transpose(1, 0, 2)).astype(bf)
    bq_h = np.ascontiguousarray((bq * sc).reshape(H, P).T).astype(np.float32)
    bkv_k = np.stack([bkv[256 * g:256 * g + 128] for g in range(G)], axis=1)
    bkv_v = np.concatenate([bkv[256 * g + 128:256 * g + 256] for g in range(G)])
    bkvk_h = np.ascontiguousarray(bkv_k).astype(np.float32)
    bkvv_h = np.ascontiguousarray(bkv_v.reshape(1, 512)).astype(np.float32)
    bo_h = np.ascontiguousarray(bo.reshape(1, E)).astype(np.float32)

    in_maps = []
    for c in range(NCORES):
        b, q = divmod(c, 4)
        xq = x[b, 512 * q:512 * (q + 1), :].T  # [e, s_own] — own quarter only
        xt_h = np.ascontiguousarray(
            xq.reshape(EB, P, SQ).transpose(1, 0, 2)).astype(bf)
        in_maps.append({"xt": xt_h, "wq": wq_h, "wkv": wkv_h, "wo": wo_h,
                        "bq": bq_h, "bkvk": bkvk_h, "bkvv": bkvv_h, "bo": bo_h})

    res = run_bass_kernel_spmd(nc, in_maps, core_ids=list(range(NCORES)),
                               trace=TRACE)
    LAST_RESULT = res

    outf = np.empty((2, S, E), np.float32)
    for c in range(NCORES):
        b, q = divmod(c, 4)
        outf[b, 512 * q:512 * (q + 1), :] = res.results[c]["out"]
    return outf


# revision 34
# speedup vs baseline: 1.0024x; 1.0024x over previous
"""Grouped Query Attention on 8 TRN2 NeuronCores (v7).

Sharding: batch x s_q-quarter (core c -> batch c//4, query rows
[512*(c%4), 512*(c%4+1))). Each core computes the Q projection for its
512 query rows, attention for all 16 heads over its query rows, and the
output projection for a disjoint [512, 2048] slice of the output.

KV: each core projects K^T and V for its OWN sequence quarter, packs
them into DRAM (per-piece, right after each evacuation), and two
4-core AllGathers (K first, then V) assemble the full K^T/V while the
tensor engine runs the Q projection.

v7 structure:
- Phase-1 PE order K -> Q0-3 -> V -> Q4-15 so the V weights (second
  half of wkv) have time to land; wkv K-columns stream first.
- DMA queues: consts+x+wq(even) on sync, wkv+packs+K-gathers on
  scalar, wq(odd)+V-gathers on gpsimd; all wq into fresh buffers (no
  WAR waits blocking the collective doorbells).
- Phase 2 is one flat software pipeline over 128 (pair, k-tile) steps:
  scores+exp (A) stream with attnV (B) lagging 3 steps; no prologue
  stall and no per-pair boundary stalls.
- Softmax denominator per pair: DVE tree-add over the 16 k-tiles +
  GPSIMD partition_all_reduce; reciprocal+normalize deferred ~1 pair.
  The last pair takes a short path (half-trees + PE ones-matmul +
  broadcast) so phase 3 is not held up.
- Bias adds via DVE tensor_add with pre-broadcast bias rows; 1/sqrt(d)
  folded into Wq on host.
"""

import numpy as np

E = 2048
S = 2048
P = 128
H = 16
G = 4
SQ = 512          # query rows per core
EB = E // P       # 16 e-blocks (contraction tiles)
NCORES = 8

_NC = None
TRACE = False
LAST_RESULT = None


def _build():
    import concourse.bacc as bacc
    import concourse.mybir as mybir
    import concourse.tile as tile
    from concourse import bass_isa

    f32 = mybir.dt.float32
    bf16 = mybir.dt.bfloat16
    EXP = mybir.ActivationFunctionType.Exp
    IDENT = mybir.ActivationFunctionType.Identity

    nc = bacc.Bacc("TRN2", target_bir_lowering=False, debug=False,
                   num_devices=NCORES)

    xt = nc.declare_dram_parameter("xt", [P, EB, SQ], bf16, isOutput=False).ap()
    wq = nc.declare_dram_parameter("wq", [H, P, EB, P], bf16, isOutput=False).ap()
    wkv = nc.declare_dram_parameter("wkv", [P, EB, 2 * E // G], bf16, isOutput=False).ap()
    wo = nc.declare_dram_parameter("wo", [P, EB, E], bf16, isOutput=False).ap()
    bq = nc.declare_dram_parameter("bq", [P, H], f32, isOutput=False).ap()
    bkvk = nc.declare_dram_parameter("bkvk", [P, 4], f32, isOutput=False).ap()
    bkvv = nc.declare_dram_parameter("bkvv", [1, 512], f32, isOutput=False).ap()
    bo = nc.declare_dram_parameter("bo", [1, E], f32, isOutput=False).ap()
    out = nc.declare_dram_parameter("out", [SQ, E], f32, isOutput=True).ap()

    RG = [[0, 1, 2, 3], [4, 5, 6, 7]]

    with tile.TileContext(nc) as tc:
        with tc.tile_pool(name="consts", bufs=1) as cp, \
             tc.tile_pool(name="qtsp", bufs=1) as qtsp, \
             tc.tile_pool(name="kvp", bufs=1) as kvp, \
             tc.tile_pool(name="otp", bufs=1) as otp, \
             tc.tile_pool(name="dram", bufs=1, space="DRAM") as dp:
            # consts on the sync queue, ahead of x (tiny transfers)
            bq_s = cp.tile([P, H], f32, tag="bqs")
            nc.sync.dma_start(bq_s, bq)
            bkvk_s = cp.tile([P, 4], f32, tag="bkvks")
            nc.sync.dma_start(bkvk_s, bkvk)
            bkvv_b = cp.tile([P, 512], f32, tag="bkvvb")
            bo_b = cp.tile([P, E], f32, tag="bob")
            onec = cp.tile([P, 1], bf16, tag="onec")
            nc.vector.memset(onec, 1.0)

            qts = qtsp.tile([P, H, SQ], bf16, tag="qts")    # Q^T, [hd, head, sq]
            kts = kvp.tile([P, G, S], bf16, tag="kts")      # K^T, [hd, group, sk]
            vgs = kvp.tile([P, EB, 512], bf16, tag="vgs")   # V, [sk, sk_tile, g*128+hd]
            OT = otp.tile([P, H, SQ], bf16, tag="ot")       # attn out, [hd, head, sq]

            kvkown = dp.tile([P, 4, 512], bf16, tag="kvkown")
            kvkall = dp.tile([4, P, 4, 512], bf16, tag="kvkall")
            kvvown = dp.tile([P, 4, 512], bf16, tag="kvvown")
            kvvall = dp.tile([4, P, 4, 512], bf16, tag="kvvall")

            # ---- Phase 1: projections from the SBUF-resident x^T.
            with tc.tile_pool(name="xsp", bufs=1) as xsp, \
                 tc.tile_pool(name="wkvp", bufs=1) as wkvp, \
                 tc.tile_pool(name="kvsg", bufs=1) as kvsg, \
                 tc.tile_pool(name="wqp", bufs=16) as wqp, \
                 tc.tile_pool(name="ps1", bufs=2, space="PSUM") as ps1, \
                 tc.tile_pool(name="ps1b", bufs=2, space="PSUM") as ps1b:
                # wkv on the scalar queue, K columns first in small
                # chunks: the K proj -> collective path is critical
                # V columns first: the V projection (and its collective)
                # runs first so vgs is ready well before the B-pass
                wkv_s = wkvp.tile([P, EB, 2 * E // G], bf16, tag="wkvs")
                KC = 2 * E // G
                for c4 in range(4):
                    nc.scalar.dma_start(
                        wkv_s[:, 4 * c4:4 * (c4 + 1), 512:KC],
                        wkv[:, 4 * c4:4 * (c4 + 1), 512:KC])
                nc.scalar.dma_start(wkv_s[:, 0:8, 0:512], wkv[:, 0:8, 0:512])
                nc.scalar.dma_start(wkv_s[:, 8:16, 0:512], wkv[:, 8:16, 0:512])
                # bias staging rows; broadcasts run on gpsimd
                bkvv_s = kvsg.tile([1, 512], f32, tag="bkvvs")
                nc.sync.dma_start(bkvv_s, bkvv)
                bo_s = kvsg.tile([1, E], f32, tag="bos")
                nc.sync.dma_start(bo_s, bo)

                # x on sync; wq split sync(even)/gpsimd(odd), all into
                # fresh buffers (no WAR waits on the gpsimd queue)
                xs = xsp.tile([P, EB, SQ], bf16, tag="xs")
                nc.sync.dma_start(xs[:, 0:2], xt[:, 0:2])
                nc.sync.dma_start(xs[:, 2:4], xt[:, 2:4])
                nc.sync.dma_start(xs[:, 4:10], xt[:, 4:10])
                nc.sync.dma_start(xs[:, 10:16], xt[:, 10:16])
                wq_s = []
                for m in range(H):
                    wqm = wqp.tile([P, EB, P], bf16, tag="wqm")
                    if m % 2 == 0:
                        nc.sync.dma_start(wqm, wq[m])
                    else:
                        nc.gpsimd.dma_start(wqm, wq[m])
                    wq_s.append(wqm)
                nc.gpsimd.partition_broadcast(bkvv_b, bkvv_s)
                nc.gpsimd.partition_broadcast(bo_b, bo_s)

                kvstg_k = kvsg.tile([P, 4, 512], bf16, tag="kvstgk")
                kvstg_v = kvsg.tile([P, 4, 512], bf16, tag="kvstgv")

                def q_head(m):
                    ps = ps1.tile([P, SQ], f32, tag="ps")
                    for b in range(EB):
                        nc.tensor.matmul(ps, wq_s[m][:, b], xs[:, b],
                                         start=(b == 0), stop=(b == EB - 1))
                    nc.vector.tensor_scalar_add(qts[:, m], ps,
                                                bq_s[:, m:m + 1])

                # V in [s, d] orientation; per-piece packs, collective
                # first so vgs lands long before the B-pass needs it
                for t in range(4):
                    ps = ps1b.tile([P, 512], f32, tag="ps")
                    for b in range(EB):
                        nc.tensor.matmul(
                            ps, xs[:, b, t * P:(t + 1) * P],
                            wkv_s[:, b, 512:KC],
                            start=(b == 0), stop=(b == EB - 1))
                    nc.vector.tensor_add(kvstg_v[:, t], ps, bkvv_b)
                    nc.scalar.dma_start(kvvown[:, t], kvstg_v[:, t])
                nc.gpsimd.collective_compute(
                    "AllGather", mybir.AluOpType.bypass,
                    replica_groups=RG, ins=[kvvown[:]], outs=[kvvall[:]])
                for c in range(4):
                    nc.gpsimd.dma_start(vgs[:, 4 * c:4 * (c + 1)],
                                        kvvall[c])

                # K^T for all 4 groups; pack each piece immediately
                for m in range(G):
                    ps = ps1b.tile([P, 512], f32, tag="ps")
                    for b in range(EB):
                        nc.tensor.matmul(
                            ps, wkv_s[:, b, m * P:(m + 1) * P], xs[:, b],
                            start=(b == 0), stop=(b == EB - 1))
                    nc.scalar.activation(kvstg_k[:, m], ps, IDENT,
                                         bias=bkvk_s[:, m:m + 1])
                    nc.scalar.dma_start(kvkown[:, m], kvstg_k[:, m])
                nc.gpsimd.collective_compute(
                    "AllGather", mybir.AluOpType.bypass,
                    replica_groups=RG, ins=[kvkown[:]], outs=[kvkall[:]])
                # K gathers on scalar (it later carries the exp stream)
                for c in range(4):
                    nc.scalar.dma_start(
                        kts[:, :, 512 * c:512 * (c + 1)], kvkall[c])

                for m in range(H):
                    q_head(m)

            # ---- Phase 2: one flat A/B software pipeline.
            with tc.tile_pool(name="wop", bufs=2) as wop, \
                 tc.tile_pool(name="eap", bufs=2) as eap, \
                 tc.tile_pool(name="r1p", bufs=1) as r1p, \
                 tc.tile_pool(name="rsp", bufs=1) as rsp, \
                 tc.tile_pool(name="dbp", bufs=1) as dbp, \
                 tc.tile_pool(name="rbp", bufs=1) as rbp, \
                 tc.tile_pool(name="psop", bufs=4, space="PSUM") as psop:
                won0 = wop.tile([P, EB, 512], bf16, tag="won")
                nc.sync.dma_start(won0, wo[:, :, 0:512])  # prefetch ph3

                recs = [None] * 8
                psos = [None] * 8
                eas = [None] * 8

                def denom_front(p):
                    # DVE tree over the 16 k-tiles -> gpsimd all-reduce
                    ea = eas[p]
                    r1 = r1p.tile([P, 8, 2, SQ], bf16, tag="r1")
                    nc.vector.tensor_add(r1, ea[:, 0:8], ea[:, 8:16])
                    nc.vector.tensor_add(r1[:, 0:4], r1[:, 0:4], r1[:, 4:8])
                    nc.vector.tensor_add(r1[:, 0:2], r1[:, 0:2], r1[:, 2:4])
                    rs = rsp.tile([P, 2, SQ], bf16, tag="rs")
                    nc.vector.tensor_add(rs, r1[:, 0], r1[:, 1])
                    den = dbp.tile([P, 2, SQ], f32, tag="den")
                    nc.gpsimd.partition_all_reduce(
                        den, rs, channels=P,
                        reduce_op=bass_isa.ReduceOp.add)
                    rec = rbp.tile([P, 2, SQ], f32, tag="rec")
                    recs[p] = (den, rec)

                def denom_back(p):
                    den, rec = recs[p]
                    nc.vector.reciprocal_approx_fast(rec, den)
                    pso0, pso1 = psos[p]
                    nc.vector.tensor_mul(OT[:, 2 * p], pso0, rec[:, 0])
                    nc.vector.tensor_mul(OT[:, 2 * p + 1], pso1, rec[:, 1])

                def denom7_half(r1, ea, lo):
                    nc.vector.tensor_add(r1[:, lo:lo + 2],
                                         ea[:, 2 * lo:2 * lo + 2],
                                         ea[:, 2 * lo + 2:2 * lo + 4])
                    nc.vector.tensor_add(r1[:, lo:lo + 1],
                                         r1[:, lo:lo + 1],
                                         r1[:, lo + 1:lo + 2])

                pscp_cm = tc.tile_pool(name="pscp", bufs=2, space="PSUM")
                pscp = pscp_cm.__enter__()
                LAG = 3
                r17 = None
                for i in range(128 + LAG):
                    if i < 128:
                        p, t = divmod(i, EB)
                        if t == 0:
                            ea_new = eap.tile([P, EB, 2, SQ], bf16, tag="ea")
                            eas[p] = ea_new
                        ps2 = pscp.tile([P, 2, SQ], f32, tag="ps2")
                        nc.tensor.matmul(ps2[:, 0],
                                         kts[:, p // 2, t * P:(t + 1) * P],
                                         qts[:, 2 * p], start=True, stop=True)
                        nc.tensor.matmul(ps2[:, 1],
                                         kts[:, p // 2, t * P:(t + 1) * P],
                                         qts[:, 2 * p + 1], start=True,
                                         stop=True)
                        nc.scalar.activation(eas[p][:, t], ps2, EXP)
                    if i >= LAG:
                        j = i - LAG
                        pb, tb = divmod(j, EB)
                        if tb == 0:
                            pso0 = psop.tile([P, SQ], f32, tag="pso")
                            pso1 = psop.tile([P, SQ], f32, tag="pso")
                            psos[pb] = (pso0, pso1)
                        pso0, pso1 = psos[pb]
                        nc.tensor.matmul(
                            pso0, vgs[:, tb, (pb // 2) * P:(pb // 2 + 1) * P],
                            eas[pb][:, tb, 0],
                            start=(tb == 0), stop=(tb == EB - 1))
                        nc.tensor.matmul(
                            pso1, vgs[:, tb, (pb // 2) * P:(pb // 2 + 1) * P],
                            eas[pb][:, tb, 1],
                            start=(tb == 0), stop=(tb == EB - 1))
                    # deferred denominator work, keyed to pipeline position
                    q16, r16 = divmod(i, EB)
                    if r16 == 2 and 1 <= q16 <= 7:
                        denom_front(q16 - 1)
                    if r16 == 15 and 1 <= q16 <= 7:
                        denom_back(q16 - 1)
                    if i == 123:
                        r17 = r1p.tile([P, 8, 2, SQ], bf16, tag="r1")
                        denom7_half(r17, eas[7], 0)
                        denom7_half(r17, eas[7], 2)
                        nc.vector.tensor_add(r17[:, 0], r17[:, 0], r17[:, 2])
                pscp_cm.__exit__(None, None, None)

                # epilogue: pair 7 short-path normalization
                denom7_half(r17, eas[7], 4)
                denom7_half(r17, eas[7], 6)
                nc.vector.tensor_add(r17[:, 4], r17[:, 4], r17[:, 6])
                rs7 = rsp.tile([P, 2, SQ], bf16, tag="rs")
                nc.vector.tensor_add(rs7, r17[:, 0], r17[:, 4])
                psl0 = psop.tile([1, 512], f32, tag="pso")
                nc.tensor.matmul(psl0, onec, rs7[:, 0], start=True, stop=True)
                psl1 = psop.tile([1, 512], f32, tag="pso")
                nc.tensor.matmul(psl1, onec, rs7[:, 1], start=True, stop=True)
                rec7s = dbp.tile([1, 2, 512], f32, tag="den")
                nc.vector.reciprocal_approx_fast(rec7s[:, 0], psl0)
                nc.vector.reciprocal_approx_fast(rec7s[:, 1], psl1)
                rec7b = rbp.tile([P, 2, 512], f32, tag="rec")
                nc.gpsimd.partition_broadcast(rec7b, rec7s)
                pso0, pso1 = psos[7]
                nc.vector.tensor_mul(OT[:, 14], pso0, rec7b[:, 0])
                nc.vector.tensor_mul(OT[:, 15], pso1, rec7b[:, 1])

                # ---- Phase 3: output projection, contraction over the
                # 16 head blocks; bias added on the PSUM->SBUF copy.
                with tc.tile_pool(name="obp", bufs=2) as obp, \
                     tc.tile_pool(name="ps3", bufs=2, space="PSUM") as ps3p:
                    wons = [won0]
                    for n in range(4):
                        if n + 1 < 4:
                            wnx = wop.tile([P, EB, 512], bf16, tag="won")
                            nc.sync.dma_start(
                                wnx, wo[:, :, 512 * (n + 1):512 * (n + 2)])
                            wons.append(wnx)
                        won = wons[n]
                        for ms in range(4):
                            ps = ps3p.tile([P, 512], f32, tag="ps")
                            for k in range(EB):
                                nc.tensor.matmul(
                                    ps, OT[:, k, ms * P:(ms + 1) * P],
                                    won[:, k],
                                    start=(k == 0), stop=(k == EB - 1))
                            ob = obp.tile([P, 512], f32, tag="ob")
                            nc.vector.tensor_add(
                                ob, ps, bo_b[:, 512 * n:512 * (n + 1)])
                            nc.sync.dma_start(
                                out[ms * P:(ms + 1) * P,
                                    512 * n:512 * (n + 1)], ob)

    nc.compile()
    return nc


def _get_nc():
    global _NC
    if _NC is None:
        _NC = _build()
    return _NC


def kernel(x, Wq, bq, Wkv, bkv, Wo, bo):
    from concourse.bass_utils import run_bass_kernel_spmd
    import ml_dtypes
    global LAST_RESULT

    bf = ml_dtypes.bfloat16
    x = np.asarray(x, np.float32)
    Wq = np.asarray(Wq, np.float32)
    bq = np.asarray(bq, np.float32)
    Wkv = np.asarray(Wkv, np.float32)
    bkv = np.asarray(bkv, np.float32)
    Wo = np.asarray(Wo, np.float32)
    bo = np.asarray(bo, np.float32)

    nc = _get_nc()
    sc = 1.0 / np.sqrt(E // H)
    wq_h = np.ascontiguousarray(
        (Wq * sc).reshape(EB, P, H, P).transpose(2, 1, 0, 3)).astype(bf)
    kcols = np.concatenate([Wkv[:, 256 * g:256 * g + 128] for g in range(G)], axis=1)
    vcols = np.concatenate([Wkv[:, 256 * g + 128:256 * g + 256] for g in range(G)], axis=1)
    wkv_re = np.concatenate([kcols, vcols], axis=1)  # [E, 1024]
    wkv_h = np.ascontiguousarray(
        wkv_re.reshape(EB, P, 2 * E // G).transpose(1, 0, 2)).astype(bf)
    wo_h = np.ascontiguousarray(Wo.reshape(EB, P, E).transpose(1, 0, 2)).astype(bf)
    bq_h = np.ascontiguousarray((bq * sc).reshape(H, P).T).astype(np.float32)
    bkv_k = np.stack([bkv[256 * g:256 * g + 128] for g in range(G)], axis=1)
    bkv_v = np.concatenate([bkv[256 * g + 128:256 * g + 256] for g in range(G)])
    bkvk_h = np.ascontiguousarray(bkv_k).astype(np.float32)
    bkvv_h = np.ascontiguousarray(bkv_v.reshape(1, 512)).astype(np.float32)
    bo_h = np.ascontiguousarray(bo.reshape(1, E)).astype(np.float32)

    in_maps = []
    for c in range(NCORES):
        b, q = divmod(c, 4)
        xq = x[b, 512 * q:512 * (q + 1), :].T  # [e, s_own] — own quarter only
        xt_h = np.ascontiguousarray(
            xq.reshape(EB, P, SQ).transpose(1, 0, 2)).astype(bf)
        in_maps.append({"xt": xt_h, "wq": wq_h, "wkv": wkv_h, "wo": wo_h,
                        "bq": bq_h, "bkvk": bkvk_h, "bkvv": bkvv_h, "bo": bo_h})

    res = run_bass_kernel_spmd(nc, in_maps, core_ids=list(range(NCORES)),
                               trace=TRACE)
    LAST_RESULT = res

    outf = np.empty((2, S, E), np.float32)
    for c in range(NCORES):
        b, q = divmod(c, 4)
        outf[b, 512 * q:512 * (q + 1), :] = res.results[c]["out"]
    return outf
